# revision 13
# baseline (speedup 1.0000x reference)
"""Trainium2 Bass kernel for nn_CrossEpochAttention (B=8192, S=10, D=1024, H=8).

Strategy: pure data parallel over 8 NeuronCores (1024 batches each), fp16
operands everywhere (host casts x and the weights), fused single pass per
480-token chunk with a fully slot-structured tensor-engine schedule:

Every head-slot pairs one dependent small matmul (score / P-transpose /
attn@V, 120 cols) with ~800ns of independent big matmuls (512/480 cols) so
the in-order engine queues never stall on the softmax DVE/Act chain:
  - A-slot: score_h | 2x V-projection | 2x next-chunk QK projection
  - B-slot: PT_h    | 2x prev-chunk Wo | 2x next-chunk QK | attn@V_h
The QK projection of chunk c+1 is dribbled 2-per-slot through chunk c
(128 matmuls == 128 slot positions per chunk, an exact fit); Wo of chunk c
streams tokens ([d_out, tok] orientation, N=480 — saves the 512/120 padding
waste of the [tok, d_out] form) and runs 2-per-B-slot inside chunk c+1
(64 matmuls == 64 B-slot positions), with its bias on the scalar engine's
per-partition bias port and the output stored fp16. Softmax normalization
(reciprocal + P*1/rowsum) is emitted right after each group's A-pass so the
B-pass DVE queue holds only the PSUM copies.
"""

import os
import sys

for _p in (
    "/root/.axon_site",
    "/root/.axon_site/_ro/trn_rl_repo",
    "/root/.axon_site/_ro/pypackages",
    "/opt/trn_rl_repo",
):
    if os.path.isdir(_p) and _p not in sys.path:
        sys.path.append(_p)

import numpy as np

import concourse.bacc as bacc
import concourse.mybir as mybir
import concourse.tile as tile
from concourse import bass_utils

F32 = mybir.dt.float32
F16 = mybir.dt.float16
AF = mybir.ActivationFunctionType
ADD = mybir.AluOpType.add

# problem constants
B, S, D = 8192, 10, 1024
H, HD = 8, 128
NCORES = 8
B_LOC = B // NCORES            # 1024
TOK = B_LOC * S                # 10240
GSZ = 120                      # 12 batches per attention group
RSZ = 40                       # runt group: 4 batches
T_CHUNK = 480                  # 4 groups per chunk
SCALE = float(HD) ** -0.5
MASK_NEG = -30000.0
KD = D // 128                  # 8 contraction tiles


def _chunks():
    """(t0, T, [(goff, gsz), ...]) per chunk; 21 full chunks + 160-token tail."""
    out = []
    t0 = 0
    while t0 + T_CHUNK <= 85 * GSZ:
        out.append((t0, T_CHUNK, [(g * GSZ, GSZ) for g in range(T_CHUNK // GSZ)]))
        t0 += T_CHUNK
    rem_groups = []
    goff = 0
    while t0 + goff < 85 * GSZ:
        rem_groups.append((goff, GSZ))
        goff += GSZ
    rem_groups.append((goff, RSZ))
    out.append((t0, goff + RSZ, rem_groups))
    assert t0 + goff + RSZ == TOK
    return out


def _build():
    nc = bacc.Bacc("TRN2", target_bir_lowering=False, debug=False,
                   num_devices=NCORES)

    xT = nc.dram_tensor("xT", [128, KD, TOK], F16, kind="ExternalInput")
    wq = nc.dram_tensor("wqT", [128, KD, D], F16, kind="ExternalInput")
    wk = nc.dram_tensor("wkT", [128, KD, D], F16, kind="ExternalInput")
    wv = nc.dram_tensor("wvT", [128, KD, D], F16, kind="ExternalInput")
    wo = nc.dram_tensor("woT", [128, KD, D], F16, kind="ExternalInput")
    bqs = nc.dram_tensor("bq_s", [128, KD], F32, kind="ExternalInput")
    bk_ = nc.dram_tensor("bk_l", [128, KD], F32, kind="ExternalInput")
    bvb = nc.dram_tensor("bv_b", [128, D], F32, kind="ExternalInput")
    bos = nc.dram_tensor("bo_s", [128, KD], F32, kind="ExternalInput")
    mkf = nc.dram_tensor("mask_full", [GSZ, GSZ], F32, kind="ExternalInput")
    mkr = nc.dram_tensor("mask_runt", [RSZ, RSZ], F32, kind="ExternalInput")
    out = nc.dram_tensor("out", [128, KD, TOK], F16, kind="ExternalOutput")

    chunks_l = _chunks()

    with tile.TileContext(nc) as tc:
        with tc.tile_pool(name="const", bufs=1) as cpool, \
             tc.tile_pool(name="w1", bufs=1) as wpool, \
             tc.tile_pool(name="xt", bufs=3) as xpool, \
             tc.tile_pool(name="qkv", bufs=2) as qkvpool, \
             tc.tile_pool(name="ot", bufs=2) as opool, \
             tc.tile_pool(name="att", bufs=4) as apool, \
             tc.tile_pool(name="pn", bufs=36) as pnpool, \
             tc.tile_pool(name="pp", bufs=36) as ppool, \
             tc.tile_pool(name="sm", bufs=10) as smpool, \
             tc.tile_pool(name="fin", bufs=4) as fpool, \
             tc.tile_pool(name="ps_qk", bufs=2, space="PSUM") as qk_ps, \
             tc.tile_pool(name="ps_vwo", bufs=2, space="PSUM") as vwo_ps, \
             tc.tile_pool(name="ps_score", bufs=2, space="PSUM") as score_ps, \
             tc.tile_pool(name="ps_pt", bufs=1, space="PSUM") as pt_ps, \
             tc.tile_pool(name="ps_ot", bufs=1, space="PSUM") as ot_ps:

            ident = cpool.tile([128, 128], F32)
            from concourse.masks import make_identity
            make_identity(nc, ident[:])
            identh = cpool.tile([128, 128], F16)
            nc.vector.tensor_copy(identh[:], ident[:])

            # DMA order = need order (single sync HWDGE queue): first the
            # j=0 QK weight slices and the first contraction tile of chunk-0
            # x so the dense QK prologue can start ASAP, then the rest in
            # first-use order.
            wq_sb = wpool.tile([128, KD, D], F16, tag="wq")
            wk_sb = wpool.tile([128, KD, D], F16, tag="wk")
            bqs_sb = cpool.tile([128, KD], F32)
            bk_sb = cpool.tile([128, KD], F32)

            xt_tiles = [None] * len(chunks_l)

            def xt_prefetch(ci, split=False):
                if ci >= len(chunks_l) or xt_tiles[ci] is not None:
                    return
                t0, T, _ = chunks_l[ci]
                xf = xpool.tile([128, KD, T_CHUNK], F16, tag="xt", name="xt")
                if split:
                    for kk in range(KD):
                        nc.sync.dma_start(xf[:, kk, :T],
                                          xT.ap()[:, kk, t0:t0 + T])
                else:
                    nc.sync.dma_start(xf[:, :, :T], xT.ap()[:, :, t0:t0 + T])
                xt_tiles[ci] = xf

            nc.sync.dma_start(wq_sb[:, :, :128], wq.ap()[:, :, :128])
            t0_, T_, _ = chunks_l[0]
            xf0 = xpool.tile([128, KD, T_CHUNK], F16, tag="xt", name="xt")
            nc.sync.dma_start(xf0[:, 0, :T_], xT.ap()[:, 0, t0_:t0_ + T_])
            nc.scalar.dma_start(wk_sb[:, :, :128], wk.ap()[:, :, :128])
            nc.scalar.dma_start(bqs_sb[:], bqs.ap())
            nc.scalar.dma_start(bk_sb[:], bk_.ap())
            for kk in range(1, KD):
                nc.sync.dma_start(xf0[:, kk, :T_], xT.ap()[:, kk, t0_:t0_ + T_])
            xt_tiles[0] = xf0
            # remaining QK weight slices: wq on the sync queue, wk on the act
            # queue, emitted inside the dense prologue chain (below) so act
            # DMAs never blockade the chain's bias activations.
            _wslices = []
            for j in range(1, KD):
                _wslices.append((nc.sync, wq_sb, wq, j))
                _wslices.append((nc.scalar, wk_sb, wk, j))
            wv_sb = wpool.tile([128, KD, D], F16, tag="wv")
            bvb_sb = cpool.tile([128, D], F32)
            mkf_sb = cpool.tile([GSZ, GSZ], F32)
            mkr_sb = cpool.tile([RSZ, RSZ], F32)
            wo_sb = wpool.tile([128, KD, D], F16, tag="wo")
            bos_sb = cpool.tile([128, KD], F32)

            def _late_prologue_dmas():
                nc.sync.dma_start(wv_sb[:, :, :512], wv.ap()[:, :, :512])
                nc.scalar.dma_start(bvb_sb[:], bvb.ap())
                nc.scalar.dma_start(mkf_sb[:], mkf.ap())
                nc.scalar.dma_start(mkr_sb[:], mkr.ap())
                nc.sync.dma_start(wv_sb[:, :, 512:], wv.ap()[:, :, 512:])
                xt_prefetch(1, split=True)
                nc.sync.dma_start(wo_sb[:], wo.ap())
                nc.scalar.dma_start(bos_sb[:], bos.ap())

            # ---- dribbled QK projection for a chunk ----
            def make_qk_state(ci):
                if ci >= len(chunks_l):
                    return None
                T = chunks_l[ci][1]
                qt = qkvpool.tile([128, KD, T_CHUNK], F16, tag="qt", name="qt")
                kt = qkvpool.tile([128, KD, T_CHUNK], F16, tag="kt", name="kt")
                return {"qt": qt, "kt": kt, "T": T, "xt": xt_tiles[ci],
                        "step": 0, "ps": None}

            def qk_step(st):
                if st is None or st["step"] >= 2 * KD * KD:
                    return
                k = st["step"]
                T = st["T"]
                j, is_k, kk = k // 16, (k % 16) >= KD, k % KD
                if kk == 0:
                    st["ps"] = qk_ps.tile([128, 512], F32, tag="qk",
                                          name="qk")[:, :T]
                w_sb = wk_sb if is_k else wq_sb
                nc.tensor.matmul(
                    st["ps"], w_sb[:, kk, j * 128:(j + 1) * 128],
                    st["xt"][:, kk, :T], start=(kk == 0), stop=(kk == KD - 1))
                if kk == KD - 1:
                    if is_k:
                        nc.scalar.activation(st["kt"][:, j, :T], st["ps"],
                                             AF.Identity,
                                             bias=bk_sb[:, j:j + 1], scale=1.0)
                    else:
                        nc.scalar.activation(st["qt"][:, j, :T], st["ps"],
                                             AF.Identity,
                                             bias=bqs_sb[:, j:j + 1],
                                             scale=SCALE)
                st["step"] += 1

            # ---- deferred Wo projection of a finished chunk ----
            # [d_out, tok] orientation: lhsT = Wo tile, rhs = ot (streams the
            # chunk's tokens), bias via the scalar engine's per-partition port.
            def wo_mm_step(p):
                if p is None or p["step"] >= KD * KD:
                    return
                k = p["step"]
                j, kk = k // KD, k % KD
                T = p["T"]
                if kk == 0:
                    p["ps"] = vwo_ps.tile([128, 512], F32, tag="vwo",
                                          name="vwo")[:, :T]
                nc.tensor.matmul(
                    p["ps"], wo_sb[:, kk, j * 128:(j + 1) * 128],
                    p["ot"][:, kk, :T], start=(kk == 0), stop=(kk == KD - 1))
                p["step"] += 1
                if p["step"] % KD == 0:
                    f_sb = fpool.tile([128, 512], F16, tag="f", name="f")[:, :T]
                    nc.scalar.activation(f_sb, p["ps"], AF.Identity,
                                         bias=bos_sb[:, j:j + 1])
                    nc.sync.dma_start(
                        out.ap()[:, j, p["t0"]:p["t0"] + T], f_sb)

            def wo_finish(p):
                if p is None:
                    return
                while p["step"] < KD * KD:
                    wo_mm_step(p)

            pending = None

            # prologue: chunk 0's QK, emitted densely with the remaining
            # weight-slice DMAs dribbled one-per-half-chain
            st_cur = make_qk_state(0)
            _si = 0
            while st_cur["step"] < 2 * KD * KD:
                qk_step(st_cur)
                if st_cur["step"] % KD == 0 and _si < len(_wslices):
                    eng, tile_, dram_, j = _wslices[_si]
                    _si += 1
                    eng.dma_start(tile_[:, :, j * 128:(j + 1) * 128],
                                  dram_.ap()[:, :, j * 128:(j + 1) * 128])
            _late_prologue_dmas()

            for ci, (t0, T, groups) in enumerate(chunks_l):
                xt = xt_tiles[ci][:, :, :T]
                qt_full, kt_full = st_cur["qt"], st_cur["kt"]
                st_next = make_qk_state(ci + 1)
                xt_prefetch(ci + 2)

                # ---- A-pass: scores + mask + exp; V and next-chunk QK fill
                v_sb = qkvpool.tile([128, T_CHUNK // GSZ, D], F16, tag="v",
                                    name="v")
                pts_all = []
                for gi, (goff, gsz) in enumerate(groups):
                    msk = mkf_sb if gsz == GSZ else mkr_sb
                    ssum = smpool.tile([GSZ, H], F32, tag="ssum",
                                       name="ssum")[:gsz]
                    p_tiles = []
                    vstate = {"fps": [None, None], "step": 0}

                    def v_mm_step(st=vstate, goff=goff, gsz=gsz, gi=gi):
                        k = st["step"]
                        hf, kk = k // KD, k % KD
                        if kk == 0:
                            st["fps"][hf] = vwo_ps.tile(
                                [128, 512], F32, tag="vwo", name="vwo")[:gsz]
                        nc.tensor.matmul(
                            st["fps"][hf], xt[:, kk, goff:goff + gsz],
                            wv_sb[:, kk, hf * 512:(hf + 1) * 512],
                            start=(kk == 0), stop=(kk == KD - 1))
                        st["step"] += 1
                        if st["step"] % KD == 0:
                            nc.vector.tensor_tensor(
                                v_sb[:gsz, gi, hf * 512:(hf + 1) * 512],
                                st["fps"][hf],
                                bvb_sb[:gsz, hf * 512:(hf + 1) * 512], ADD)

                    for h in range(H):
                        sps = score_ps.tile([GSZ, GSZ], F32, tag="score",
                                            name="score")[:gsz, :gsz]
                        nc.tensor.matmul(sps, qt_full[:, h, goff:goff + gsz],
                                         kt_full[:, h, goff:goff + gsz],
                                         start=True, stop=True)
                        v_mm_step()
                        v_mm_step()
                        qk_step(st_next)
                        qk_step(st_next)
                        a_sb = apool.tile([GSZ, GSZ], F32, tag="a",
                                          name="a")[:gsz, :gsz]
                        nc.vector.tensor_tensor(a_sb, sps, msk[:gsz, :gsz], ADD)
                        p_sb = ppool.tile([GSZ, GSZ], F32, tag="p",
                                          name="p")[:gsz, :gsz]
                        nc.scalar.activation(p_sb, a_sb, AF.Exp,
                                             accum_out=ssum[:, h:h + 1])
                        p_tiles.append(p_sb)
                    # normalization prep for this group (DVE) — emitted here so
                    # the B-pass DVE queue holds only the PSUM copies. The
                    # LAST group's normalizes are deferred into the first
                    # B-group's slots so the A->B boundary doesn't queue the
                    # tensor engine behind an 8-op DVE burst.
                    rs = smpool.tile([GSZ, H], F32, tag="rs", name="rs")[:gsz]
                    nc.vector.reciprocal(rs, ssum)
                    pns = []
                    for h in range(H):
                        pn = pnpool.tile([GSZ, GSZ], F16, tag="pn",
                                         name="pn")[:gsz, :gsz]
                        pns.append(pn)
                    if gi < len(groups) - 1:
                        for h in range(H):
                            nc.vector.tensor_scalar_mul(pns[h], p_tiles[h],
                                                        rs[:, h:h + 1])
                    else:
                        deferred_norm = (pns, p_tiles, rs)
                    pts_all.append(pns)

                # ---- B-pass: transpose + attn@V; prev-chunk Wo and next-chunk
                # QK fill each slot
                ot_sb = opool.tile([128, KD, T_CHUNK], F16, tag="ot",
                                   name="ot")[:, :, :T]
                for gi, (goff, gsz) in enumerate(groups):
                    pns = pts_all[gi]
                    for h in range(H):
                        ptp = pt_ps.tile([GSZ, GSZ], F16, tag="ptp",
                                         name="ptp")[:gsz, :gsz]
                        nc.tensor.transpose(ptp, pns[h], identh[:gsz, :gsz])
                        qk_step(st_next)
                        qk_step(st_next)
                        wo_mm_step(pending)
                        wo_mm_step(pending)
                        if st_next is None:
                            # runt chunk: no next-chunk QK filler, so dribble
                            # the pending Wo twice as fast
                            wo_mm_step(pending)
                            wo_mm_step(pending)
                        pt_sb = apool.tile([GSZ, GSZ], F16, tag="pt",
                                           name="pt")[:gsz, :gsz]
                        nc.vector.tensor_copy(pt_sb, ptp)
                        if gi == 0:
                            dn_pns, dn_p, dn_rs = deferred_norm
                            nc.vector.tensor_scalar_mul(dn_pns[h], dn_p[h],
                                                        dn_rs[:, h:h + 1])
                        otp = ot_ps.tile([128, GSZ], F32, tag="otp",
                                         name="otp")[:, :gsz]
                        nc.tensor.matmul(
                            otp, v_sb[:gsz, gi, h * 128:(h + 1) * 128],
                            pt_sb, start=True, stop=True)
                        nc.vector.tensor_copy(ot_sb[:, h, goff:goff + gsz], otp)

                wo_finish(pending)
                pending = {"ot": ot_sb, "t0": t0, "T": T, "step": 0,
                           "ps": None}
                st_cur = st_next

            wo_finish(pending)

    nc.compile()
    return nc


_NC = None


def _get_nc():
    global _NC
    if _NC is None:
        _NC = _build()
    return _NC


def _mask(pos_bias, nb):
    """Additive mask [nb*S, nb*S]: pos_bias[k-q+S-1] on the block diagonal,
    MASK_NEG off it."""
    n = nb * S
    q = np.arange(n)
    k = np.arange(n)
    same = (q[:, None] // S) == (k[None, :] // S)
    rel = (k[None, :] % S) - (q[:, None] % S) + S - 1
    m = np.where(same, pos_bias[rel], np.float32(MASK_NEG))
    return np.ascontiguousarray(m, np.float32)


def _in_maps(x, Wq, bq, Wk, bk, Wv, bv, Wo, bo, pos_bias):
    x = np.asarray(x, np.float32)

    def wlay(w):  # [d_out, d_in] -> [p, kk, d_out] fp16 with d_in = kk*128+p
        return np.ascontiguousarray(
            np.asarray(w, np.float32).T.reshape(KD, 128, D)
            .transpose(1, 0, 2)).astype(np.float16)

    def blay(b):  # [d_out] -> [p, j] with d_out = j*128+p
        return np.ascontiguousarray(np.asarray(b, np.float32).reshape(KD, 128).T)

    common = {
        "wqT": wlay(Wq), "wkT": wlay(Wk), "wvT": wlay(Wv), "woT": wlay(Wo),
        "bq_s": blay(np.asarray(bq, np.float32) * np.float32(SCALE)),
        "bk_l": blay(bk),
        "bv_b": np.ascontiguousarray(
            np.broadcast_to(np.asarray(bv, np.float32), (128, D))),
        "bo_s": blay(bo),
        "mask_full": _mask(np.asarray(pos_bias, np.float32), GSZ // S),
        "mask_runt": _mask(np.asarray(pos_bias, np.float32), RSZ // S),
    }
    in_maps = []
    for i in range(NCORES):
        xs = x[i * B_LOC:(i + 1) * B_LOC].reshape(TOK, D)
        xTl = np.ascontiguousarray(
            xs.T.reshape(KD, 128, TOK).transpose(1, 0, 2)).astype(np.float16)
        in_maps.append({"xT": xTl, **common})
    return in_maps


def _gather(res):
    """[128, KD, TOK] fp16 per core -> [B, S, D] fp32."""
    outs = []
    for i in range(NCORES):
        arr = res.results[i]["out"]            # [128, KD, TOK]
        outs.append(np.ascontiguousarray(arr.transpose(2, 1, 0))
                    .reshape(B_LOC, S, D).astype(np.float32))
    return np.concatenate(outs, axis=0)


def kernel(x, Wq, bq, Wk, bk, Wv, bv, Wo, bo, pos_bias):
    nc = _get_nc()
    in_maps = _in_maps(x, Wq, bq, Wk, bk, Wv, bv, Wo, bo, pos_bias)

    res = bass_utils.run_bass_kernel_spmd(nc, in_maps,
                                          core_ids=list(range(NCORES)))
    return _gather(res)


# revision 16
# speedup vs baseline: 1.0005x; 1.0005x over previous
"""Trainium2 Bass kernel for nn_CrossEpochAttention (B=8192, S=10, D=1024, H=8).

Strategy: pure data parallel over 8 NeuronCores (1024 batches each), fp16
operands everywhere (host casts x and the weights), fused single pass per
480-token chunk with a fully slot-structured tensor-engine schedule:

Every head-slot pairs one dependent small matmul (score / P-transpose /
attn@V, 120 cols) with ~800ns of independent big matmuls (512/480 cols) so
the in-order engine queues never stall on the softmax DVE/Act chain:
  - A-slot: score_h | 2x V-projection | 2x next-chunk QK projection
  - B-slot: PT_h    | 2x prev-chunk Wo | 2x next-chunk QK | attn@V_h
The QK projection of chunk c+1 is dribbled 2-per-slot through chunk c
(128 matmuls == 128 slot positions per chunk, an exact fit); Wo of chunk c
streams tokens ([d_out, tok] orientation, N=480 — saves the 512/120 padding
waste of the [tok, d_out] form) and runs 2-per-B-slot inside chunk c+1
(64 matmuls == 64 B-slot positions), with its bias on the scalar engine's
per-partition bias port and the output stored fp16. Softmax normalization
(reciprocal + P*1/rowsum) is emitted right after each group's A-pass so the
B-pass DVE queue holds only the PSUM copies.
"""

import os
import sys

for _p in (
    "/root/.axon_site",
    "/root/.axon_site/_ro/trn_rl_repo",
    "/root/.axon_site/_ro/pypackages",
    "/opt/trn_rl_repo",
):
    if os.path.isdir(_p) and _p not in sys.path:
        sys.path.append(_p)

import numpy as np

import concourse.bacc as bacc
import concourse.mybir as mybir
import concourse.tile as tile
from concourse import bass_utils

F32 = mybir.dt.float32
F16 = mybir.dt.float16
AF = mybir.ActivationFunctionType
ADD = mybir.AluOpType.add

# problem constants
B, S, D = 8192, 10, 1024
H, HD = 8, 128
NCORES = 8
B_LOC = B // NCORES            # 1024
TOK = B_LOC * S                # 10240
GSZ = 120                      # 12 batches per attention group
RSZ = 40                       # runt group: 4 batches
T_CHUNK = 480                  # 4 groups per chunk
SCALE = float(HD) ** -0.5
MASK_NEG = -30000.0
KD = D // 128                  # 8 contraction tiles


def _chunks():
    """(t0, T, [(goff, gsz), ...]) per chunk; 21 full chunks + 160-token tail."""
    out = []
    t0 = 0
    while t0 + T_CHUNK <= 85 * GSZ:
        out.append((t0, T_CHUNK, [(g * GSZ, GSZ) for g in range(T_CHUNK // GSZ)]))
        t0 += T_CHUNK
    rem_groups = []
    goff = 0
    while t0 + goff < 85 * GSZ:
        rem_groups.append((goff, GSZ))
        goff += GSZ
    rem_groups.append((goff, RSZ))
    out.append((t0, goff + RSZ, rem_groups))
    assert t0 + goff + RSZ == TOK
    return out


def _build():
    nc = bacc.Bacc("TRN2", target_bir_lowering=False, debug=False,
                   num_devices=NCORES)

    xT = nc.dram_tensor("xT", [128, KD, TOK], F16, kind="ExternalInput")
    wq = nc.dram_tensor("wqT", [128, KD, D], F16, kind="ExternalInput")
    wk = nc.dram_tensor("wkT", [128, KD, D], F16, kind="ExternalInput")
    wv = nc.dram_tensor("wvT", [128, KD, D], F16, kind="ExternalInput")
    wo = nc.dram_tensor("woT", [128, KD, D], F16, kind="ExternalInput")
    bqs = nc.dram_tensor("bq_s", [128, KD], F32, kind="ExternalInput")
    bk_ = nc.dram_tensor("bk_l", [128, KD], F32, kind="ExternalInput")
    bvb = nc.dram_tensor("bv_b", [128, D], F32, kind="ExternalInput")
    bos = nc.dram_tensor("bo_s", [128, KD], F32, kind="ExternalInput")
    mkf = nc.dram_tensor("mask_full", [GSZ, GSZ], F32, kind="ExternalInput")
    mkr = nc.dram_tensor("mask_runt", [RSZ, RSZ], F32, kind="ExternalInput")
    out = nc.dram_tensor("out", [128, KD, TOK], F16, kind="ExternalOutput")

    chunks_l = _chunks()

    with tile.TileContext(nc) as tc:
        with tc.tile_pool(name="const", bufs=1) as cpool, \
             tc.tile_pool(name="w1", bufs=1) as wpool, \
             tc.tile_pool(name="xt", bufs=3) as xpool, \
             tc.tile_pool(name="qkv", bufs=2) as qkvpool, \
             tc.tile_pool(name="ot", bufs=2) as opool, \
             tc.tile_pool(name="att", bufs=4) as apool, \
             tc.tile_pool(name="pn", bufs=36) as pnpool, \
             tc.tile_pool(name="pp", bufs=36) as ppool, \
             tc.tile_pool(name="sm", bufs=10) as smpool, \
             tc.tile_pool(name="fin", bufs=4) as fpool, \
             tc.tile_pool(name="ps_qk", bufs=2, space="PSUM") as qk_ps, \
             tc.tile_pool(name="ps_vwo", bufs=2, space="PSUM") as vwo_ps, \
             tc.tile_pool(name="ps_score", bufs=2, space="PSUM") as score_ps, \
             tc.tile_pool(name="ps_pt", bufs=1, space="PSUM") as pt_ps, \
             tc.tile_pool(name="ps_ot", bufs=1, space="PSUM") as ot_ps:

            ident = cpool.tile([128, 128], F32)
            from concourse.masks import make_identity
            make_identity(nc, ident[:])
            identh = cpool.tile([128, 128], F16)
            nc.vector.tensor_copy(identh[:], ident[:])

            # DMA order = need order (single sync HWDGE queue): first the
            # j=0 QK weight slices and the first contraction tile of chunk-0
            # x so the dense QK prologue can start ASAP, then the rest in
            # first-use order.
            wq_sb = wpool.tile([128, KD, D], F16, tag="wq")
            wk_sb = wpool.tile([128, KD, D], F16, tag="wk")
            bqs_sb = cpool.tile([128, KD], F32)
            bk_sb = cpool.tile([128, KD], F32)

            xt_tiles = [None] * len(chunks_l)

            def xt_prefetch(ci, split=False):
                if ci >= len(chunks_l) or xt_tiles[ci] is not None:
                    return
                t0, T, _ = chunks_l[ci]
                xf = xpool.tile([128, KD, T_CHUNK], F16, tag="xt", name="xt")
                if split:
                    for kk in range(KD):
                        nc.sync.dma_start(xf[:, kk, :T],
                                          xT.ap()[:, kk, t0:t0 + T])
                else:
                    nc.sync.dma_start(xf[:, :, :T], xT.ap()[:, :, t0:t0 + T])
                xt_tiles[ci] = xf

            nc.sync.dma_start(wq_sb[:, :, :128], wq.ap()[:, :, :128])
            t0_, T_, _ = chunks_l[0]
            xf0 = xpool.tile([128, KD, T_CHUNK], F16, tag="xt", name="xt")
            nc.sync.dma_start(xf0[:, 0, :T_], xT.ap()[:, 0, t0_:t0_ + T_])
            nc.scalar.dma_start(wk_sb[:, :, :128], wk.ap()[:, :, :128])
            nc.scalar.dma_start(bqs_sb[:], bqs.ap())
            nc.scalar.dma_start(bk_sb[:], bk_.ap())
            for kk in range(1, KD):
                nc.sync.dma_start(xf0[:, kk, :T_], xT.ap()[:, kk, t0_:t0_ + T_])
            xt_tiles[0] = xf0
            # remaining QK weight slices: wq on the sync queue, wk on the act
            # queue, emitted inside the dense prologue chain (below) so act
            # DMAs never blockade the chain's bias activations.
            _wslices = []
            for j in range(1, KD):
                _wslices.append((nc.sync, wq_sb, wq, j))
                _wslices.append((nc.scalar, wk_sb, wk, j))
            wv_sb = wpool.tile([128, KD, D], F16, tag="wv")
            bvb_sb = cpool.tile([128, D], F32)
            mkf_sb = cpool.tile([GSZ, GSZ], F32)
            mkr_sb = cpool.tile([RSZ, RSZ], F32)
            wo_sb = wpool.tile([128, KD, D], F16, tag="wo")
            bos_sb = cpool.tile([128, KD], F32)

            def _late_prologue_dmas():
                nc.sync.dma_start(wv_sb[:, :, :512], wv.ap()[:, :, :512])
                nc.scalar.dma_start(bvb_sb[:], bvb.ap())
                nc.scalar.dma_start(mkf_sb[:], mkf.ap())
                nc.scalar.dma_start(mkr_sb[:], mkr.ap())
                xt_prefetch(1, split=True)
                nc.sync.dma_start(wv_sb[:, :, 512:], wv.ap()[:, :, 512:])
                nc.sync.dma_start(wo_sb[:], wo.ap())
                nc.scalar.dma_start(bos_sb[:], bos.ap())

            # ---- dribbled QK projection for a chunk ----
            def make_qk_state(ci):
                if ci >= len(chunks_l):
                    return None
                T = chunks_l[ci][1]
                qt = qkvpool.tile([128, KD, T_CHUNK], F16, tag="qt", name="qt")
                kt = qkvpool.tile([128, KD, T_CHUNK], F16, tag="kt", name="kt")
                return {"qt": qt, "kt": kt, "T": T, "xt": xt_tiles[ci],
                        "step": 0, "ps": None}

            def qk_step(st):
                if st is None or st["step"] >= 2 * KD * KD:
                    return
                k = st["step"]
                T = st["T"]
                j, is_k, kk = k // 16, (k % 16) >= KD, k % KD
                if kk == 0:
                    st["ps"] = qk_ps.tile([128, 512], F32, tag="qk",
                                          name="qk")[:, :T]
                w_sb = wk_sb if is_k else wq_sb
                nc.tensor.matmul(
                    st["ps"], w_sb[:, kk, j * 128:(j + 1) * 128],
                    st["xt"][:, kk, :T], start=(kk == 0), stop=(kk == KD - 1))
                if kk == KD - 1:
                    if is_k:
                        nc.scalar.activation(st["kt"][:, j, :T], st["ps"],
                                             AF.Identity,
                                             bias=bk_sb[:, j:j + 1], scale=1.0)
                    else:
                        nc.scalar.activation(st["qt"][:, j, :T], st["ps"],
                                             AF.Identity,
                                             bias=bqs_sb[:, j:j + 1],
                                             scale=SCALE)
                st["step"] += 1

            # ---- deferred Wo projection of a finished chunk ----
            # [d_out, tok] orientation: lhsT = Wo tile, rhs = ot (streams the
            # chunk's tokens), bias via the scalar engine's per-partition port.
            def wo_mm_step(p):
                if p is None or p["step"] >= KD * KD:
                    return
                k = p["step"]
                j, kk = k // KD, k % KD
                T = p["T"]
                if kk == 0:
                    p["ps"] = vwo_ps.tile([128, 512], F32, tag="vwo",
                                          name="vwo")[:, :T]
                nc.tensor.matmul(
                    p["ps"], wo_sb[:, kk, j * 128:(j + 1) * 128],
                    p["ot"][:, kk, :T], start=(kk == 0), stop=(kk == KD - 1))
                p["step"] += 1
                if p["step"] % KD == 0:
                    f_sb = fpool.tile([128, 512], F16, tag="f", name="f")[:, :T]
                    nc.scalar.activation(f_sb, p["ps"], AF.Identity,
                                         bias=bos_sb[:, j:j + 1])
                    nc.sync.dma_start(
                        out.ap()[:, j, p["t0"]:p["t0"] + T], f_sb)

            def wo_finish(p):
                if p is None:
                    return
                while p["step"] < KD * KD:
                    wo_mm_step(p)

            pending = None

            # prologue: chunk 0's QK, emitted densely with the remaining
            # weight-slice DMAs dribbled one-per-half-chain
            st_cur = make_qk_state(0)
            _si = 0
            while st_cur["step"] < 2 * KD * KD:
                qk_step(st_cur)
                if st_cur["step"] % KD == 0 and _si < len(_wslices):
                    eng, tile_, dram_, j = _wslices[_si]
                    _si += 1
                    eng.dma_start(tile_[:, :, j * 128:(j + 1) * 128],
                                  dram_.ap()[:, :, j * 128:(j + 1) * 128])
            _late_prologue_dmas()

            for ci, (t0, T, groups) in enumerate(chunks_l):
                xt = xt_tiles[ci][:, :, :T]
                qt_full, kt_full = st_cur["qt"], st_cur["kt"]
                st_next = make_qk_state(ci + 1)
                xt_prefetch(ci + 2)

                # ---- A-pass: scores + mask + exp; V and next-chunk QK fill
                v_sb = qkvpool.tile([128, T_CHUNK // GSZ, D], F16, tag="v",
                                    name="v")
                pts_all = []
                for gi, (goff, gsz) in enumerate(groups):
                    msk = mkf_sb if gsz == GSZ else mkr_sb
                    ssum = smpool.tile([GSZ, H], F32, tag="ssum",
                                       name="ssum")[:gsz]
                    p_tiles = []
                    vstate = {"fps": [None, None], "step": 0}

                    def v_mm_step(st=vstate, goff=goff, gsz=gsz, gi=gi):
                        k = st["step"]
                        hf, kk = k // KD, k % KD
                        if kk == 0:
                            st["fps"][hf] = vwo_ps.tile(
                                [128, 512], F32, tag="vwo", name="vwo")[:gsz]
                        nc.tensor.matmul(
                            st["fps"][hf], xt[:, kk, goff:goff + gsz],
                            wv_sb[:, kk, hf * 512:(hf + 1) * 512],
                            start=(kk == 0), stop=(kk == KD - 1))
                        st["step"] += 1
                        if st["step"] % KD == 0:
                            nc.vector.tensor_tensor(
                                v_sb[:gsz, gi, hf * 512:(hf + 1) * 512],
                                st["fps"][hf],
                                bvb_sb[:gsz, hf * 512:(hf + 1) * 512], ADD)

                    for h in range(H):
                        sps = score_ps.tile([GSZ, GSZ], F32, tag="score",
                                            name="score")[:gsz, :gsz]
                        nc.tensor.matmul(sps, qt_full[:, h, goff:goff + gsz],
                                         kt_full[:, h, goff:goff + gsz],
                                         start=True, stop=True)
                        v_mm_step()
                        v_mm_step()
                        qk_step(st_next)
                        qk_step(st_next)
                        a_sb = apool.tile([GSZ, GSZ], F32, tag="a",
                                          name="a")[:gsz, :gsz]
                        nc.vector.tensor_tensor(a_sb, sps, msk[:gsz, :gsz], ADD)
                        p_sb = ppool.tile([GSZ, GSZ], F32, tag="p",
                                          name="p")[:gsz, :gsz]
                        nc.scalar.activation(p_sb, a_sb, AF.Exp,
                                             accum_out=ssum[:, h:h + 1])
                        p_tiles.append(p_sb)
                    # normalization prep for this group (DVE) — emitted here so
                    # the B-pass DVE queue holds only the PSUM copies. The
                    # LAST group's normalizes are deferred into the first
                    # B-group's slots so the A->B boundary doesn't queue the
                    # tensor engine behind an 8-op DVE burst.
                    rs = smpool.tile([GSZ, H], F32, tag="rs", name="rs")[:gsz]
                    nc.vector.reciprocal(rs, ssum)
                    pns = []
                    for h in range(H):
                        pn = pnpool.tile([GSZ, GSZ], F16, tag="pn",
                                         name="pn")[:gsz, :gsz]
                        pns.append(pn)
                    if gi < len(groups) - 1:
                        for h in range(H):
                            nc.vector.tensor_scalar_mul(pns[h], p_tiles[h],
                                                        rs[:, h:h + 1])
                    else:
                        deferred_norm = (pns, p_tiles, rs)
                    pts_all.append(pns)

                # ---- B-pass: transpose + attn@V; prev-chunk Wo and next-chunk
                # QK fill each slot
                ot_sb = opool.tile([128, KD, T_CHUNK], F16, tag="ot",
                                   name="ot")[:, :, :T]
                for gi, (goff, gsz) in enumerate(groups):
                    pns = pts_all[gi]
                    for h in range(H):
                        ptp = pt_ps.tile([GSZ, GSZ], F16, tag="ptp",
                                         name="ptp")[:gsz, :gsz]
                        nc.tensor.transpose(ptp, pns[h], identh[:gsz, :gsz])
                        qk_step(st_next)
                        qk_step(st_next)
                        wo_mm_step(pending)
                        wo_mm_step(pending)
                        if st_next is None:
                            # runt chunk: no next-chunk QK filler, so dribble
                            # the pending Wo twice as fast
                            wo_mm_step(pending)
                            wo_mm_step(pending)
                        pt_sb = apool.tile([GSZ, GSZ], F16, tag="pt",
                                           name="pt")[:gsz, :gsz]
                        nc.vector.tensor_copy(pt_sb, ptp)
                        if gi == 0:
                            dn_pns, dn_p, dn_rs = deferred_norm
                            nc.vector.tensor_scalar_mul(dn_pns[h], dn_p[h],
                                                        dn_rs[:, h:h + 1])
                        otp = ot_ps.tile([128, GSZ], F32, tag="otp",
                                         name="otp")[:, :gsz]
                        nc.tensor.matmul(
                            otp, v_sb[:gsz, gi, h * 128:(h + 1) * 128],
                            pt_sb, start=True, stop=True)
                        nc.vector.tensor_copy(ot_sb[:, h, goff:goff + gsz], otp)

                wo_finish(pending)
                pending = {"ot": ot_sb, "t0": t0, "T": T, "step": 0,
                           "ps": None}
                st_cur = st_next

            wo_finish(pending)

    nc.compile()
    return nc


_NC = None


def _get_nc():
    global _NC
    if _NC is None:
        _NC = _build()
    return _NC


def _mask(pos_bias, nb):
    """Additive mask [nb*S, nb*S]: pos_bias[k-q+S-1] on the block diagonal,
    MASK_NEG off it."""
    n = nb * S
    q = np.arange(n)
    k = np.arange(n)
    same = (q[:, None] // S) == (k[None, :] // S)
    rel = (k[None, :] % S) - (q[:, None] % S) + S - 1
    m = np.where(same, pos_bias[rel], np.float32(MASK_NEG))
    return np.ascontiguousarray(m, np.float32)


def _in_maps(x, Wq, bq, Wk, bk, Wv, bv, Wo, bo, pos_bias):
    x = np.asarray(x, np.float32)

    def wlay(w):  # [d_out, d_in] -> [p, kk, d_out] fp16 with d_in = kk*128+p
        return np.ascontiguousarray(
            np.asarray(w, np.float32).T.reshape(KD, 128, D)
            .transpose(1, 0, 2)).astype(np.float16)

    def blay(b):  # [d_out] -> [p, j] with d_out = j*128+p
        return np.ascontiguousarray(np.asarray(b, np.float32).reshape(KD, 128).T)

    common = {
        "wqT": wlay(Wq), "wkT": wlay(Wk), "wvT": wlay(Wv), "woT": wlay(Wo),
        "bq_s": blay(np.asarray(bq, np.float32) * np.float32(SCALE)),
        "bk_l": blay(bk),
        "bv_b": np.ascontiguousarray(
            np.broadcast_to(np.asarray(bv, np.float32), (128, D))),
        "bo_s": blay(bo),
        "mask_full": _mask(np.asarray(pos_bias, np.float32), GSZ // S),
        "mask_runt": _mask(np.asarray(pos_bias, np.float32), RSZ // S),
    }
    in_maps = []
    for i in range(NCORES):
        xs = x[i * B_LOC:(i + 1) * B_LOC].reshape(TOK, D)
        xTl = np.ascontiguousarray(
            xs.T.reshape(KD, 128, TOK).transpose(1, 0, 2)).astype(np.float16)
        in_maps.append({"xT": xTl, **common})
    return in_maps


def _gather(res):
    """[128, KD, TOK] fp16 per core -> [B, S, D] fp32."""
    outs = []
    for i in range(NCORES):
        arr = res.results[i]["out"]            # [128, KD, TOK]
        outs.append(np.ascontiguousarray(arr.transpose(2, 1, 0))
                    .reshape(B_LOC, S, D).astype(np.float32))
    return np.concatenate(outs, axis=0)


def kernel(x, Wq, bq, Wk, bk, Wv, bv, Wo, bo, pos_bias):
    nc = _get_nc()
    in_maps = _in_maps(x, Wq, bq, Wk, bk, Wv, bv, Wo, bo, pos_bias)

    res = bass_utils.run_bass_kernel_spmd(nc, in_maps,
                                          core_ids=list(range(NCORES)))
    return _gather(res)


# revision 19
# speedup vs baseline: 1.0047x; 1.0042x over previous
"""Trainium2 Bass kernel for nn_CrossEpochAttention (B=8192, S=10, D=1024, H=8).

Strategy: pure data parallel over 8 NeuronCores (1024 batches each), fp16
operands everywhere (host casts x and the weights), fused single pass per
480-token chunk with a fully slot-structured tensor-engine schedule:

Every head-slot pairs one dependent small matmul (score / P-transpose /
attn@V, 120 cols) with ~800ns of independent big matmuls (512/480 cols) so
the in-order engine queues never stall on the softmax DVE/Act chain:
  - A-slot: score_h | 2x V-projection | 2x next-chunk QK projection
  - B-slot: PT_h    | 2x prev-chunk Wo | 2x next-chunk QK | attn@V_h
The QK projection of chunk c+1 is dribbled 2-per-slot through chunk c
(128 matmuls == 128 slot positions per chunk, an exact fit); Wo of chunk c
streams tokens ([d_out, tok] orientation, N=480 — saves the 512/120 padding
waste of the [tok, d_out] form) and runs 2-per-B-slot inside chunk c+1
(64 matmuls == 64 B-slot positions), with its bias on the scalar engine's
per-partition bias port and the output stored fp16. Softmax normalization
(reciprocal + P*1/rowsum) is emitted right after each group's A-pass so the
B-pass DVE queue holds only the PSUM copies.
"""

import os
import sys

for _p in (
    "/root/.axon_site",
    "/root/.axon_site/_ro/trn_rl_repo",
    "/root/.axon_site/_ro/pypackages",
    "/opt/trn_rl_repo",
):
    if os.path.isdir(_p) and _p not in sys.path:
        sys.path.append(_p)

import numpy as np

import concourse.bacc as bacc
import concourse.mybir as mybir
import concourse.tile as tile
from concourse import bass_utils

F32 = mybir.dt.float32
F16 = mybir.dt.float16
AF = mybir.ActivationFunctionType
ADD = mybir.AluOpType.add

# problem constants
B, S, D = 8192, 10, 1024
H, HD = 8, 128
NCORES = 8
B_LOC = B // NCORES            # 1024
TOK = B_LOC * S                # 10240
GSZ = 120                      # 12 batches per attention group
RSZ = 40                       # runt group: 4 batches
T_CHUNK = 480                  # 4 groups per chunk
SCALE = float(HD) ** -0.5
MASK_NEG = -30000.0
KD = D // 128                  # 8 contraction tiles


def _chunks():
    """(t0, T, [(goff, gsz), ...]) per chunk; 21 full chunks + 160-token tail."""
    out = []
    t0 = 0
    while t0 + T_CHUNK <= 85 * GSZ:
        out.append((t0, T_CHUNK, [(g * GSZ, GSZ) for g in range(T_CHUNK // GSZ)]))
        t0 += T_CHUNK
    rem_groups = []
    goff = 0
    while t0 + goff < 85 * GSZ:
        rem_groups.append((goff, GSZ))
        goff += GSZ
    rem_groups.append((goff, RSZ))
    out.append((t0, goff + RSZ, rem_groups))
    assert t0 + goff + RSZ == TOK
    return out


def _build():
    nc = bacc.Bacc("TRN2", target_bir_lowering=False, debug=False,
                   num_devices=NCORES)

    xT = nc.dram_tensor("xT", [128, KD, TOK], F16, kind="ExternalInput")
    wq = nc.dram_tensor("wqT", [128, KD, D], F16, kind="ExternalInput")
    wk = nc.dram_tensor("wkT", [128, KD, D], F16, kind="ExternalInput")
    wv = nc.dram_tensor("wvT", [128, KD, D], F16, kind="ExternalInput")
    wo = nc.dram_tensor("woT", [128, KD, D], F16, kind="ExternalInput")
    bqs = nc.dram_tensor("bq_s", [128, KD], F32, kind="ExternalInput")
    bk_ = nc.dram_tensor("bk_l", [128, KD], F32, kind="ExternalInput")
    bvb = nc.dram_tensor("bv_b", [128, D], F32, kind="ExternalInput")
    bos = nc.dram_tensor("bo_s", [128, KD], F32, kind="ExternalInput")
    mkf = nc.dram_tensor("mask_full", [GSZ, GSZ], F32, kind="ExternalInput")
    mkr = nc.dram_tensor("mask_runt", [RSZ, RSZ], F32, kind="ExternalInput")
    out = nc.dram_tensor("out", [128, KD, TOK], F16, kind="ExternalOutput")

    chunks_l = _chunks()

    with tile.TileContext(nc) as tc:
        with tc.tile_pool(name="const", bufs=1) as cpool, \
             tc.tile_pool(name="w1", bufs=1) as wpool, \
             tc.tile_pool(name="xt", bufs=3) as xpool, \
             tc.tile_pool(name="qkv", bufs=2) as qkvpool, \
             tc.tile_pool(name="ot", bufs=2) as opool, \
             tc.tile_pool(name="att", bufs=4) as apool, \
             tc.tile_pool(name="pn", bufs=36) as pnpool, \
             tc.tile_pool(name="pp", bufs=36) as ppool, \
             tc.tile_pool(name="sm", bufs=10) as smpool, \
             tc.tile_pool(name="fin", bufs=4) as fpool, \
             tc.tile_pool(name="ps_qk", bufs=2, space="PSUM") as qk_ps, \
             tc.tile_pool(name="ps_vwo", bufs=2, space="PSUM") as vwo_ps, \
             tc.tile_pool(name="ps_score", bufs=2, space="PSUM") as score_ps, \
             tc.tile_pool(name="ps_pt", bufs=1, space="PSUM") as pt_ps, \
             tc.tile_pool(name="ps_ot", bufs=1, space="PSUM") as ot_ps:

            ident = cpool.tile([128, 128], F32)
            from concourse.masks import make_identity
            make_identity(nc, ident[:])
            identh = cpool.tile([128, 128], F16)
            nc.vector.tensor_copy(identh[:], ident[:])

            # DMA order = need order (single sync HWDGE queue): first the
            # j=0 QK weight slices and the first contraction tile of chunk-0
            # x so the dense QK prologue can start ASAP, then the rest in
            # first-use order.
            wq_sb = wpool.tile([128, KD, D], F16, tag="wq")
            wk_sb = wpool.tile([128, KD, D], F16, tag="wk")
            bqs_sb = cpool.tile([128, KD], F32)
            bk_sb = cpool.tile([128, KD], F32)

            xt_tiles = [None] * len(chunks_l)

            def xt_prefetch(ci, split=False):
                if ci >= len(chunks_l) or xt_tiles[ci] is not None:
                    return
                t0, T, _ = chunks_l[ci]
                xf = xpool.tile([128, KD, T_CHUNK], F16, tag="xt", name="xt")
                if split:
                    for kk in range(KD):
                        nc.sync.dma_start(xf[:, kk, :T],
                                          xT.ap()[:, kk, t0:t0 + T])
                else:
                    nc.sync.dma_start(xf[:, :, :T], xT.ap()[:, :, t0:t0 + T])
                xt_tiles[ci] = xf

            nc.sync.dma_start(wq_sb[:, :, :128], wq.ap()[:, :, :128])
            t0_, T_, _ = chunks_l[0]
            xf0 = xpool.tile([128, KD, T_CHUNK], F16, tag="xt", name="xt")
            nc.sync.dma_start(xf0[:, 0, :T_], xT.ap()[:, 0, t0_:t0_ + T_])
            nc.scalar.dma_start(wk_sb[:, :, :128], wk.ap()[:, :, :128])
            nc.scalar.dma_start(bqs_sb[:], bqs.ap())
            nc.scalar.dma_start(bk_sb[:], bk_.ap())
            for kk in range(1, KD):
                nc.sync.dma_start(xf0[:, kk, :T_], xT.ap()[:, kk, t0_:t0_ + T_])
            xt_tiles[0] = xf0
            # remaining QK weight slices: wq on the sync queue, wk on the act
            # queue, emitted inside the dense prologue chain (below) so act
            # DMAs never blockade the chain's bias activations.
            _wslices = []
            for j in range(1, KD):
                _wslices.append((nc.sync, wq_sb, wq, j))
                _wslices.append((nc.scalar, wk_sb, wk, j))
            wv_sb = wpool.tile([128, KD, D], F16, tag="wv")
            bvb_sb = cpool.tile([128, D], F32)
            mkf_sb = cpool.tile([GSZ, GSZ], F32)
            mkr_sb = cpool.tile([RSZ, RSZ], F32)
            wo_sb = wpool.tile([128, KD, D], F16, tag="wo")
            bos_sb = cpool.tile([128, KD], F32)

            def _late_prologue_dmas():
                nc.sync.dma_start(wv_sb[:, :, :512], wv.ap()[:, :, :512])
                nc.scalar.dma_start(bvb_sb[:], bvb.ap())
                nc.scalar.dma_start(mkf_sb[:], mkf.ap())
                nc.scalar.dma_start(mkr_sb[:], mkr.ap())
                xt_prefetch(1, split=True)
                nc.sync.dma_start(wv_sb[:, :, 512:], wv.ap()[:, :, 512:])
                nc.sync.dma_start(wo_sb[:], wo.ap())
                nc.scalar.dma_start(bos_sb[:], bos.ap())

            # ---- dribbled QK projection for a chunk ----
            def make_qk_state(ci):
                if ci >= len(chunks_l):
                    return None
                T = chunks_l[ci][1]
                qt = qkvpool.tile([128, KD, T_CHUNK], F16, tag="qt", name="qt")
                kt = qkvpool.tile([128, KD, T_CHUNK], F16, tag="kt", name="kt")
                return {"qt": qt, "kt": kt, "T": T, "xt": xt_tiles[ci],
                        "step": 0, "ps": None}

            def qk_step(st):
                if st is None or st["step"] >= 2 * KD * KD:
                    return
                k = st["step"]
                T = st["T"]
                j, is_k, kk = k // 16, (k % 16) >= KD, k % KD
                if kk == 0:
                    st["ps"] = qk_ps.tile([128, 512], F32, tag="qk",
                                          name="qk")[:, :T]
                w_sb = wk_sb if is_k else wq_sb
                nc.tensor.matmul(
                    st["ps"], w_sb[:, kk, j * 128:(j + 1) * 128],
                    st["xt"][:, kk, :T], start=(kk == 0), stop=(kk == KD - 1))
                if kk == KD - 1:
                    if is_k:
                        nc.scalar.activation(st["kt"][:, j, :T], st["ps"],
                                             AF.Identity,
                                             bias=bk_sb[:, j:j + 1], scale=1.0)
                    else:
                        nc.scalar.activation(st["qt"][:, j, :T], st["ps"],
                                             AF.Identity,
                                             bias=bqs_sb[:, j:j + 1],
                                             scale=SCALE)
                st["step"] += 1

            # ---- deferred Wo projection of a finished chunk ----
            # [d_out, tok] orientation: lhsT = Wo tile, rhs = ot (streams the
            # chunk's tokens), bias via the scalar engine's per-partition port.
            def wo_mm_step(p):
                if p is None or p["step"] >= KD * KD:
                    return
                k = p["step"]
                j, kk = k // KD, k % KD
                T = p["T"]
                if kk == 0:
                    p["ps"] = vwo_ps.tile([128, 512], F32, tag="vwo",
                                          name="vwo")[:, :T]
                nc.tensor.matmul(
                    p["ps"], wo_sb[:, kk, j * 128:(j + 1) * 128],
                    p["ot"][:, kk, :T], start=(kk == 0), stop=(kk == KD - 1))
                p["step"] += 1
                if p["step"] % KD == 0:
                    f_sb = fpool.tile([128, 512], F16, tag="f", name="f")[:, :T]
                    nc.scalar.activation(f_sb, p["ps"], AF.Identity,
                                         bias=bos_sb[:, j:j + 1])
                    nc.sync.dma_start(
                        out.ap()[:, j, p["t0"]:p["t0"] + T], f_sb)

            def wo_finish(p):
                if p is None:
                    return
                while p["step"] < KD * KD:
                    wo_mm_step(p)

            pending = None

            # softmax-normalize FIFO: drained 1-per-A-slot (first 6 slots of
            # groups after the producer) and 1-per-B-slot, so the DVE queue
            # never bursts 8 normalizes at a group boundary. FIFO order plus
            # these drain rates puts every norm ahead of its P-transpose.
            norm_q = []

            def drain_norm():
                if norm_q:
                    pn, p_sb, rs, h = norm_q.pop(0)
                    nc.vector.tensor_scalar_mul(pn, p_sb, rs[:, h:h + 1])

            # prologue: chunk 0's QK, emitted densely with the remaining
            # weight-slice DMAs dribbled one-per-half-chain
            st_cur = make_qk_state(0)
            _si = 0
            while st_cur["step"] < 2 * KD * KD:
                qk_step(st_cur)
                if st_cur["step"] % KD == 0 and _si < len(_wslices):
                    eng, tile_, dram_, j = _wslices[_si]
                    _si += 1
                    eng.dma_start(tile_[:, :, j * 128:(j + 1) * 128],
                                  dram_.ap()[:, :, j * 128:(j + 1) * 128])
            _late_prologue_dmas()

            for ci, (t0, T, groups) in enumerate(chunks_l):
                xt = xt_tiles[ci][:, :, :T]
                qt_full, kt_full = st_cur["qt"], st_cur["kt"]
                st_next = make_qk_state(ci + 1)
                xt_prefetch(ci + 2)

                # ---- A-pass: scores + mask + exp; V and next-chunk QK fill
                v_sb = qkvpool.tile([128, T_CHUNK // GSZ, D], F16, tag="v",
                                    name="v")
                pts_all = []
                for gi, (goff, gsz) in enumerate(groups):
                    msk = mkf_sb if gsz == GSZ else mkr_sb
                    ssum = smpool.tile([GSZ, H], F32, tag="ssum",
                                       name="ssum")[:gsz]
                    p_tiles = []
                    vstate = {"fps": [None, None], "step": 0}

                    def v_mm_step(st=vstate, goff=goff, gsz=gsz, gi=gi):
                        k = st["step"]
                        hf, kk = k // KD, k % KD
                        if kk == 0:
                            st["fps"][hf] = vwo_ps.tile(
                                [128, 512], F32, tag="vwo", name="vwo")[:gsz]
                        nc.tensor.matmul(
                            st["fps"][hf], xt[:, kk, goff:goff + gsz],
                            wv_sb[:, kk, hf * 512:(hf + 1) * 512],
                            start=(kk == 0), stop=(kk == KD - 1))
                        st["step"] += 1
                        if st["step"] % KD == 0:
                            nc.vector.tensor_tensor(
                                v_sb[:gsz, gi, hf * 512:(hf + 1) * 512],
                                st["fps"][hf],
                                bvb_sb[:gsz, hf * 512:(hf + 1) * 512], ADD)

                    for h in range(H):
                        sps = score_ps.tile([GSZ, GSZ], F32, tag="score",
                                            name="score")[:gsz, :gsz]
                        nc.tensor.matmul(sps, qt_full[:, h, goff:goff + gsz],
                                         kt_full[:, h, goff:goff + gsz],
                                         start=True, stop=True)
                        v_mm_step()
                        v_mm_step()
                        qk_step(st_next)
                        qk_step(st_next)
                        a_sb = apool.tile([GSZ, GSZ], F32, tag="a",
                                          name="a")[:gsz, :gsz]
                        nc.vector.tensor_tensor(a_sb, sps, msk[:gsz, :gsz], ADD)
                        if h < 6:
                            drain_norm()
                        p_sb = ppool.tile([GSZ, GSZ], F32, tag="p",
                                          name="p")[:gsz, :gsz]
                        nc.scalar.activation(p_sb, a_sb, AF.Exp,
                                             accum_out=ssum[:, h:h + 1])
                        p_tiles.append(p_sb)
                    rs = smpool.tile([GSZ, H], F32, tag="rs", name="rs")[:gsz]
                    nc.vector.reciprocal(rs, ssum)
                    pns = []
                    for h in range(H):
                        pn = pnpool.tile([GSZ, GSZ], F16, tag="pn",
                                         name="pn")[:gsz, :gsz]
                        pns.append(pn)
                        norm_q.append((pn, p_tiles[h], rs, h))
                    pts_all.append(pns)

                # ---- B-pass: transpose + attn@V; prev-chunk Wo and next-chunk
                # QK fill each slot
                ot_sb = opool.tile([128, KD, T_CHUNK], F16, tag="ot",
                                   name="ot")[:, :, :T]
                for gi, (goff, gsz) in enumerate(groups):
                    pns = pts_all[gi]
                    for h in range(H):
                        ptp = pt_ps.tile([GSZ, GSZ], F16, tag="ptp",
                                         name="ptp")[:gsz, :gsz]
                        nc.tensor.transpose(ptp, pns[h], identh[:gsz, :gsz])
                        qk_step(st_next)
                        qk_step(st_next)
                        wo_mm_step(pending)
                        wo_mm_step(pending)
                        if st_next is None:
                            # runt chunk: no next-chunk QK filler, so dribble
                            # the pending Wo twice as fast
                            wo_mm_step(pending)
                            wo_mm_step(pending)
                        pt_sb = apool.tile([GSZ, GSZ], F16, tag="pt",
                                           name="pt")[:gsz, :gsz]
                        nc.vector.tensor_copy(pt_sb, ptp)
                        drain_norm()
                        otp = ot_ps.tile([128, GSZ], F32, tag="otp",
                                         name="otp")[:, :gsz]
                        nc.tensor.matmul(
                            otp, v_sb[:gsz, gi, h * 128:(h + 1) * 128],
                            pt_sb, start=True, stop=True)
                        nc.vector.tensor_copy(ot_sb[:, h, goff:goff + gsz], otp)

                wo_finish(pending)
                pending = {"ot": ot_sb, "t0": t0, "T": T, "step": 0,
                           "ps": None}
                st_cur = st_next

            wo_finish(pending)

    nc.compile()
    return nc


_NC = None


def _get_nc():
    global _NC
    if _NC is None:
        _NC = _build()
    return _NC


def _mask(pos_bias, nb):
    """Additive mask [nb*S, nb*S]: pos_bias[k-q+S-1] on the block diagonal,
    MASK_NEG off it."""
    n = nb * S
    q = np.arange(n)
    k = np.arange(n)
    same = (q[:, None] // S) == (k[None, :] // S)
    rel = (k[None, :] % S) - (q[:, None] % S) + S - 1
    m = np.where(same, pos_bias[rel], np.float32(MASK_NEG))
    return np.ascontiguousarray(m, np.float32)


def _in_maps(x, Wq, bq, Wk, bk, Wv, bv, Wo, bo, pos_bias):
    x = np.asarray(x, np.float32)

    def wlay(w):  # [d_out, d_in] -> [p, kk, d_out] fp16 with d_in = kk*128+p
        return np.ascontiguousarray(
            np.asarray(w, np.float32).T.reshape(KD, 128, D)
            .transpose(1, 0, 2)).astype(np.float16)

    def blay(b):  # [d_out] -> [p, j] with d_out = j*128+p
        return np.ascontiguousarray(np.asarray(b, np.float32).reshape(KD, 128).T)

    common = {
        "wqT": wlay(Wq), "wkT": wlay(Wk), "wvT": wlay(Wv), "woT": wlay(Wo),
        "bq_s": blay(np.asarray(bq, np.float32) * np.float32(SCALE)),
        "bk_l": blay(bk),
        "bv_b": np.ascontiguousarray(
            np.broadcast_to(np.asarray(bv, np.float32), (128, D))),
        "bo_s": blay(bo),
        "mask_full": _mask(np.asarray(pos_bias, np.float32), GSZ // S),
        "mask_runt": _mask(np.asarray(pos_bias, np.float32), RSZ // S),
    }
    in_maps = []
    for i in range(NCORES):
        xs = x[i * B_LOC:(i + 1) * B_LOC].reshape(TOK, D)
        xTl = np.ascontiguousarray(
            xs.T.reshape(KD, 128, TOK).transpose(1, 0, 2)).astype(np.float16)
        in_maps.append({"xT": xTl, **common})
    return in_maps


def _gather(res):
    """[128, KD, TOK] fp16 per core -> [B, S, D] fp32."""
    outs = []
    for i in range(NCORES):
        arr = res.results[i]["out"]            # [128, KD, TOK]
        outs.append(np.ascontiguousarray(arr.transpose(2, 1, 0))
                    .reshape(B_LOC, S, D).astype(np.float32))
    return np.concatenate(outs, axis=0)


def kernel(x, Wq, bq, Wk, bk, Wv, bv, Wo, bo, pos_bias):
    nc = _get_nc()
    in_maps = _in_maps(x, Wq, bq, Wk, bk, Wv, bv, Wo, bo, pos_bias)

    res = bass_utils.run_bass_kernel_spmd(nc, in_maps,
                                          core_ids=list(range(NCORES)))
    return _gather(res)


# revision 24
# speedup vs baseline: 1.0049x; 1.0003x over previous
"""Trainium2 Bass kernel for nn_CrossEpochAttention (B=8192, S=10, D=1024, H=8).

Strategy: pure data parallel over 8 NeuronCores (1024 batches each), fp16
operands everywhere (host casts x and the weights), fused single pass per
480-token chunk with a fully slot-structured tensor-engine schedule:

Every head-slot pairs one dependent small matmul (score / P-transpose /
attn@V, 120 cols) with ~800ns of independent big matmuls (512/480 cols) so
the in-order engine queues never stall on the softmax DVE/Act chain:
  - A-slot: score_h | 2x V-projection | 2x next-chunk QK projection
  - B-slot: PT_h    | 2x prev-chunk Wo | 2x next-chunk QK | attn@V_h
The QK projection of chunk c+1 is dribbled 2-per-slot through chunk c
(128 matmuls == 128 slot positions per chunk, an exact fit); Wo of chunk c
streams tokens ([d_out, tok] orientation, N=480 — saves the 512/120 padding
waste of the [tok, d_out] form) and runs 2-per-B-slot inside chunk c+1
(64 matmuls == 64 B-slot positions), with its bias on the scalar engine's
per-partition bias port and the output stored fp16. Softmax normalization
(reciprocal + P*1/rowsum) is emitted right after each group's A-pass so the
B-pass DVE queue holds only the PSUM copies.
"""

import os
import sys

for _p in (
    "/root/.axon_site",
    "/root/.axon_site/_ro/trn_rl_repo",
    "/root/.axon_site/_ro/pypackages",
    "/opt/trn_rl_repo",
):
    if os.path.isdir(_p) and _p not in sys.path:
        sys.path.append(_p)

import numpy as np

import concourse.bacc as bacc
import concourse.mybir as mybir
import concourse.tile as tile
from concourse import bass_utils

F32 = mybir.dt.float32
F16 = mybir.dt.float16
AF = mybir.ActivationFunctionType
ADD = mybir.AluOpType.add

# problem constants
B, S, D = 8192, 10, 1024
H, HD = 8, 128
NCORES = 8
B_LOC = B // NCORES            # 1024
TOK = B_LOC * S                # 10240
GSZ = 120                      # 12 batches per attention group
RSZ = 40                       # runt group: 4 batches
T_CHUNK = 480                  # 4 groups per chunk
SCALE = float(HD) ** -0.5
MASK_NEG = -30000.0
KD = D // 128                  # 8 contraction tiles


def _chunks():
    """(t0, T, [(goff, gsz), ...]) per chunk; 21 full chunks + 160-token tail."""
    out = []
    t0 = 0
    while t0 + T_CHUNK <= 85 * GSZ:
        out.append((t0, T_CHUNK, [(g * GSZ, GSZ) for g in range(T_CHUNK // GSZ)]))
        t0 += T_CHUNK
    rem_groups = []
    goff = 0
    while t0 + goff < 85 * GSZ:
        rem_groups.append((goff, GSZ))
        goff += GSZ
    rem_groups.append((goff, RSZ))
    out.append((t0, goff + RSZ, rem_groups))
    assert t0 + goff + RSZ == TOK
    return out


def _build():
    nc = bacc.Bacc("TRN2", target_bir_lowering=False, debug=False,
                   num_devices=NCORES)

    xT = nc.dram_tensor("xT", [128, KD, TOK], F16, kind="ExternalInput")
    wq = nc.dram_tensor("wqT", [128, KD, D], F16, kind="ExternalInput")
    wk = nc.dram_tensor("wkT", [128, KD, D], F16, kind="ExternalInput")
    wv = nc.dram_tensor("wvT", [128, KD, D], F16, kind="ExternalInput")
    wo = nc.dram_tensor("woT", [128, KD, D], F16, kind="ExternalInput")
    bqs = nc.dram_tensor("bq_s", [128, KD], F32, kind="ExternalInput")
    bk_ = nc.dram_tensor("bk_l", [128, KD], F32, kind="ExternalInput")
    bvb = nc.dram_tensor("bv_b", [128, D], F32, kind="ExternalInput")
    bos = nc.dram_tensor("bo_s", [128, KD], F32, kind="ExternalInput")
    mkf = nc.dram_tensor("mask_full", [GSZ, GSZ], F32, kind="ExternalInput")
    mkr = nc.dram_tensor("mask_runt", [RSZ, RSZ], F32, kind="ExternalInput")
    out = nc.dram_tensor("out", [128, KD, TOK], F16, kind="ExternalOutput")

    chunks_l = _chunks()

    with tile.TileContext(nc) as tc:
        with tc.tile_pool(name="const", bufs=1) as cpool, \
             tc.tile_pool(name="w1", bufs=1) as wpool, \
             tc.tile_pool(name="xt", bufs=3) as xpool, \
             tc.tile_pool(name="qkv", bufs=2) as qkvpool, \
             tc.tile_pool(name="ot", bufs=2) as opool, \
             tc.tile_pool(name="att", bufs=4) as apool, \
             tc.tile_pool(name="pn", bufs=36) as pnpool, \
             tc.tile_pool(name="pp", bufs=36) as ppool, \
             tc.tile_pool(name="sm", bufs=10) as smpool, \
             tc.tile_pool(name="fin", bufs=4) as fpool, \
             tc.tile_pool(name="ps_qk", bufs=2, space="PSUM") as qk_ps, \
             tc.tile_pool(name="ps_vwo", bufs=2, space="PSUM") as vwo_ps, \
             tc.tile_pool(name="ps_score", bufs=2, space="PSUM") as score_ps, \
             tc.tile_pool(name="ps_pt", bufs=1, space="PSUM") as pt_ps, \
             tc.tile_pool(name="ps_ot", bufs=1, space="PSUM") as ot_ps:

            ident = cpool.tile([128, 128], F32)
            from concourse.masks import make_identity
            make_identity(nc, ident[:])
            identh = cpool.tile([128, 128], F16)
            nc.vector.tensor_copy(identh[:], ident[:])

            # DMA order = need order (single sync HWDGE queue): first the
            # j=0 QK weight slices and the first contraction tile of chunk-0
            # x so the dense QK prologue can start ASAP, then the rest in
            # first-use order.
            wq_sb = wpool.tile([128, KD, D], F16, tag="wq")
            wk_sb = wpool.tile([128, KD, D], F16, tag="wk")
            bqs_sb = cpool.tile([128, KD], F32)
            bk_sb = cpool.tile([128, KD], F32)

            xt_tiles = [None] * len(chunks_l)

            def xt_prefetch(ci, split=False):
                if ci >= len(chunks_l) or xt_tiles[ci] is not None:
                    return
                t0, T, _ = chunks_l[ci]
                xf = xpool.tile([128, KD, T_CHUNK], F16, tag="xt", name="xt")
                if split:
                    for kk in range(KD):
                        nc.sync.dma_start(xf[:, kk, :T],
                                          xT.ap()[:, kk, t0:t0 + T])
                else:
                    nc.sync.dma_start(xf[:, :, :T], xT.ap()[:, :, t0:t0 + T])
                xt_tiles[ci] = xf

            # wq/wk use a j-major layout ([128, j, kk*128+c]) so each j-slice
            # is one fully contiguous 2KB-per-partition DMA.
            nc.sync.dma_start(wq_sb[:, 0, :], wq.ap()[:, 0, :])
            t0_, T_, _ = chunks_l[0]
            xf0 = xpool.tile([128, KD, T_CHUNK], F16, tag="xt", name="xt")
            nc.sync.dma_start(xf0[:, 0, :T_], xT.ap()[:, 0, t0_:t0_ + T_])
            nc.scalar.dma_start(wk_sb[:, 0, :], wk.ap()[:, 0, :])
            nc.scalar.dma_start(bqs_sb[:], bqs.ap())
            nc.scalar.dma_start(bk_sb[:], bk_.ap())
            for kk in range(1, KD):
                nc.sync.dma_start(xf0[:, kk, :T_], xT.ap()[:, kk, t0_:t0_ + T_])
            xt_tiles[0] = xf0
            # remaining QK weight slices: wq on the sync queue, wk on the act
            # queue, emitted inside the dense prologue chain (below) so act
            # DMAs never blockade the chain's bias activations.
            _wslices = []
            for j in range(1, KD):
                _wslices.append((nc.sync, wq_sb, wq, j))
                _wslices.append((nc.scalar, wk_sb, wk, j))
            wv_sb = wpool.tile([128, KD, D], F16, tag="wv")
            bvb_sb = cpool.tile([128, D], F32)
            mkf_sb = cpool.tile([GSZ, GSZ], F32)
            mkr_sb = cpool.tile([RSZ, RSZ], F32)
            wo_sb = wpool.tile([128, KD, D], F16, tag="wo")
            bos_sb = cpool.tile([128, KD], F32)

            def _late_prologue_dmas():
                nc.sync.dma_start(wv_sb[:, :, :512], wv.ap()[:, :, :512])
                nc.scalar.dma_start(bvb_sb[:], bvb.ap())
                nc.scalar.dma_start(mkf_sb[:], mkf.ap())
                nc.scalar.dma_start(mkr_sb[:], mkr.ap())
                xt_prefetch(1, split=True)
                nc.sync.dma_start(wv_sb[:, :, 512:], wv.ap()[:, :, 512:])
                nc.sync.dma_start(wo_sb[:], wo.ap())
                nc.scalar.dma_start(bos_sb[:], bos.ap())

            # ---- dribbled QK projection for a chunk ----
            def make_qk_state(ci):
                if ci >= len(chunks_l):
                    return None
                T = chunks_l[ci][1]
                qt = qkvpool.tile([128, KD, T_CHUNK], F16, tag="qt", name="qt")
                kt = qkvpool.tile([128, KD, T_CHUNK], F16, tag="kt", name="kt")
                return {"qt": qt, "kt": kt, "T": T, "xt": xt_tiles[ci],
                        "step": 0, "ps": None}

            def qk_step(st):
                if st is None or st["step"] >= 2 * KD * KD:
                    return
                k = st["step"]
                T = st["T"]
                j, is_k, kk = k // 16, (k % 16) >= KD, k % KD
                if kk == 0:
                    st["ps"] = qk_ps.tile([128, 512], F32, tag="qk",
                                          name="qk")[:, :T]
                w_sb = wk_sb if is_k else wq_sb
                nc.tensor.matmul(
                    st["ps"], w_sb[:, j, kk * 128:(kk + 1) * 128],
                    st["xt"][:, kk, :T], start=(kk == 0), stop=(kk == KD - 1))
                if kk == KD - 1:
                    if is_k:
                        nc.scalar.activation(st["kt"][:, j, :T], st["ps"],
                                             AF.Identity,
                                             bias=bk_sb[:, j:j + 1], scale=1.0)
                    else:
                        nc.scalar.activation(st["qt"][:, j, :T], st["ps"],
                                             AF.Identity,
                                             bias=bqs_sb[:, j:j + 1],
                                             scale=SCALE)
                st["step"] += 1

            # ---- deferred Wo projection of a finished chunk ----
            # [d_out, tok] orientation: lhsT = Wo tile, rhs = ot (streams the
            # chunk's tokens), bias via the scalar engine's per-partition port.
            def wo_mm_step(p):
                if p is None or p["step"] >= KD * KD:
                    return
                k = p["step"]
                j, kk = k // KD, k % KD
                T = p["T"]
                if kk == 0:
                    p["ps"] = vwo_ps.tile([128, 512], F32, tag="vwo",
                                          name="vwo")[:, :T]
                nc.tensor.matmul(
                    p["ps"], wo_sb[:, kk, j * 128:(j + 1) * 128],
                    p["ot"][:, kk, :T], start=(kk == 0), stop=(kk == KD - 1))
                p["step"] += 1
                if p["step"] % KD == 0:
                    f_sb = fpool.tile([128, 512], F16, tag="f", name="f")[:, :T]
                    nc.scalar.activation(f_sb, p["ps"], AF.Identity,
                                         bias=bos_sb[:, j:j + 1])
                    nc.sync.dma_start(
                        out.ap()[:, j, p["t0"]:p["t0"] + T], f_sb)

            def wo_finish(p):
                if p is None:
                    return
                while p["step"] < KD * KD:
                    wo_mm_step(p)

            pending = None

            # softmax-normalize FIFO: drained 1-per-A-slot (first 6 slots of
            # groups after the producer) and 1-per-B-slot, so the DVE queue
            # never bursts 8 normalizes at a group boundary. FIFO order plus
            # these drain rates puts every norm ahead of its P-transpose.
            norm_q = []

            def drain_norm():
                if norm_q:
                    pn, p_sb, rs, h = norm_q.pop(0)
                    nc.vector.tensor_scalar_mul(pn, p_sb, rs[:, h:h + 1])

            # prologue: chunk 0's QK, emitted densely with the remaining
            # weight-slice DMAs dribbled one-per-half-chain
            st_cur = make_qk_state(0)
            _si = 0
            while st_cur["step"] < 2 * KD * KD:
                qk_step(st_cur)
                if st_cur["step"] % 4 == 0 and _si < len(_wslices):
                    eng, tile_, dram_, j = _wslices[_si]
                    _si += 1
                    eng.dma_start(tile_[:, j, :], dram_.ap()[:, j, :])
            _late_prologue_dmas()

            for ci, (t0, T, groups) in enumerate(chunks_l):
                xt = xt_tiles[ci][:, :, :T]
                qt_full, kt_full = st_cur["qt"], st_cur["kt"]
                st_next = make_qk_state(ci + 1)
                xt_prefetch(ci + 2)

                # ---- A-pass: scores + mask + exp; V and next-chunk QK fill
                v_sb = qkvpool.tile([128, T_CHUNK // GSZ, D], F16, tag="v",
                                    name="v")
                pts_all = []
                for gi, (goff, gsz) in enumerate(groups):
                    msk = mkf_sb if gsz == GSZ else mkr_sb
                    ssum = smpool.tile([GSZ, H], F32, tag="ssum",
                                       name="ssum")[:gsz]
                    p_tiles = []
                    vstate = {"fps": [None, None], "step": 0}

                    def v_mm_step(st=vstate, goff=goff, gsz=gsz, gi=gi):
                        k = st["step"]
                        hf, kk = k // KD, k % KD
                        if kk == 0:
                            st["fps"][hf] = vwo_ps.tile(
                                [128, 512], F32, tag="vwo", name="vwo")[:gsz]
                        nc.tensor.matmul(
                            st["fps"][hf], xt[:, kk, goff:goff + gsz],
                            wv_sb[:, kk, hf * 512:(hf + 1) * 512],
                            start=(kk == 0), stop=(kk == KD - 1))
                        st["step"] += 1
                        if st["step"] % KD == 0:
                            nc.vector.tensor_tensor(
                                v_sb[:gsz, gi, hf * 512:(hf + 1) * 512],
                                st["fps"][hf],
                                bvb_sb[:gsz, hf * 512:(hf + 1) * 512], ADD)

                    for h in range(H):
                        sps = score_ps.tile([GSZ, GSZ], F32, tag="score",
                                            name="score")[:gsz, :gsz]
                        nc.tensor.matmul(sps, qt_full[:, h, goff:goff + gsz],
                                         kt_full[:, h, goff:goff + gsz],
                                         start=True, stop=True)
                        v_mm_step()
                        v_mm_step()
                        qk_step(st_next)
                        qk_step(st_next)
                        a_sb = apool.tile([GSZ, GSZ], F32, tag="a",
                                          name="a")[:gsz, :gsz]
                        nc.vector.tensor_tensor(a_sb, sps, msk[:gsz, :gsz], ADD)
                        if h < 6:
                            drain_norm()
                        p_sb = ppool.tile([GSZ, GSZ], F32, tag="p",
                                          name="p")[:gsz, :gsz]
                        nc.scalar.activation(p_sb, a_sb, AF.Exp,
                                             accum_out=ssum[:, h:h + 1])
                        p_tiles.append(p_sb)
                    rs = smpool.tile([GSZ, H], F32, tag="rs", name="rs")[:gsz]
                    nc.vector.reciprocal(rs, ssum)
                    pns = []
                    for h in range(H):
                        pn = pnpool.tile([GSZ, GSZ], F16, tag="pn",
                                         name="pn")[:gsz, :gsz]
                        pns.append(pn)
                        norm_q.append((pn, p_tiles[h], rs, h))
                    pts_all.append(pns)

                # ---- B-pass: transpose + attn@V; prev-chunk Wo and next-chunk
                # QK fill each slot
                ot_sb = opool.tile([128, KD, T_CHUNK], F16, tag="ot",
                                   name="ot")[:, :, :T]
                for gi, (goff, gsz) in enumerate(groups):
                    pns = pts_all[gi]
                    for h in range(H):
                        ptp = pt_ps.tile([GSZ, GSZ], F16, tag="ptp",
                                         name="ptp")[:gsz, :gsz]
                        nc.tensor.transpose(ptp, pns[h], identh[:gsz, :gsz])
                        qk_step(st_next)
                        qk_step(st_next)
                        wo_mm_step(pending)
                        wo_mm_step(pending)
                        if st_next is None:
                            # runt chunk: no next-chunk QK filler, so dribble
                            # the pending Wo twice as fast
                            wo_mm_step(pending)
                            wo_mm_step(pending)
                        pt_sb = apool.tile([GSZ, GSZ], F16, tag="pt",
                                           name="pt")[:gsz, :gsz]
                        nc.vector.tensor_copy(pt_sb, ptp)
                        drain_norm()
                        otp = ot_ps.tile([128, GSZ], F32, tag="otp",
                                         name="otp")[:, :gsz]
                        nc.tensor.matmul(
                            otp, v_sb[:gsz, gi, h * 128:(h + 1) * 128],
                            pt_sb, start=True, stop=True)
                        nc.vector.tensor_copy(ot_sb[:, h, goff:goff + gsz], otp)

                wo_finish(pending)
                pending = {"ot": ot_sb, "t0": t0, "T": T, "step": 0,
                           "ps": None}
                st_cur = st_next

            wo_finish(pending)

    nc.compile()
    return nc


_NC = None


def _get_nc():
    global _NC
    if _NC is None:
        _NC = _build()
    return _NC


def _mask(pos_bias, nb):
    """Additive mask [nb*S, nb*S]: pos_bias[k-q+S-1] on the block diagonal,
    MASK_NEG off it."""
    n = nb * S
    q = np.arange(n)
    k = np.arange(n)
    same = (q[:, None] // S) == (k[None, :] // S)
    rel = (k[None, :] % S) - (q[:, None] % S) + S - 1
    m = np.where(same, pos_bias[rel], np.float32(MASK_NEG))
    return np.ascontiguousarray(m, np.float32)


def _in_maps(x, Wq, bq, Wk, bk, Wv, bv, Wo, bo, pos_bias):
    x = np.asarray(x, np.float32)

    def wlay(w):  # [d_out, d_in] -> [p, kk, d_out] fp16 with d_in = kk*128+p
        return np.ascontiguousarray(
            np.asarray(w, np.float32).T.reshape(KD, 128, D)
            .transpose(1, 0, 2)).astype(np.float16)

    def wlay_j(w):  # [d_out, d_in] -> [p, j, kk*128+c] with d_out = j*128+c
        a = np.asarray(w, np.float32).reshape(KD, 128, KD, 128)
        return np.ascontiguousarray(
            a.transpose(3, 0, 2, 1).reshape(128, KD, D)).astype(np.float16)

    def blay(b):  # [d_out] -> [p, j] with d_out = j*128+p
        return np.ascontiguousarray(np.asarray(b, np.float32).reshape(KD, 128).T)

    common = {
        "wqT": wlay_j(Wq), "wkT": wlay_j(Wk), "wvT": wlay(Wv), "woT": wlay(Wo),
        "bq_s": blay(np.asarray(bq, np.float32) * np.float32(SCALE)),
        "bk_l": blay(bk),
        "bv_b": np.ascontiguousarray(
            np.broadcast_to(np.asarray(bv, np.float32), (128, D))),
        "bo_s": blay(bo),
        "mask_full": _mask(np.asarray(pos_bias, np.float32), GSZ // S),
        "mask_runt": _mask(np.asarray(pos_bias, np.float32), RSZ // S),
    }
    in_maps = []
    for i in range(NCORES):
        xs = x[i * B_LOC:(i + 1) * B_LOC].reshape(TOK, D)
        xTl = np.ascontiguousarray(
            xs.T.reshape(KD, 128, TOK).transpose(1, 0, 2)).astype(np.float16)
        in_maps.append({"xT": xTl, **common})
    return in_maps


def _gather(res):
    """[128, KD, TOK] fp16 per core -> [B, S, D] fp32."""
    outs = []
    for i in range(NCORES):
        arr = res.results[i]["out"]            # [128, KD, TOK]
        outs.append(np.ascontiguousarray(arr.transpose(2, 1, 0))
                    .reshape(B_LOC, S, D).astype(np.float32))
    return np.concatenate(outs, axis=0)


def kernel(x, Wq, bq, Wk, bk, Wv, bv, Wo, bo, pos_bias):
    nc = _get_nc()
    in_maps = _in_maps(x, Wq, bq, Wk, bk, Wv, bv, Wo, bo, pos_bias)

    res = bass_utils.run_bass_kernel_spmd(nc, in_maps,
                                          core_ids=list(range(NCORES)))
    return _gather(res)
